# revision 15
# baseline (speedup 1.0000x reference)
"""Causal depthwise conv1d (K=7) over [B=4, T=4096, C=2048] f32, on 8 TRN2 cores.

Strategy:
- Tensor-parallel over channels: core i owns C/8 = 256 channels (zero comm).
- Host-side relayout to [B, C, T] so channels sit on SBUF partitions and time is
  the free axis (the HBM-natural [T, C] layout cannot be loaded channel-major
  efficiently: 4-byte strided DMA).
- Per 128-channel block, the conv is 7 PSUM-accumulated matmuls with a diagonal
  stationary matrix diag(w[:, k]) and the moving operand being the same SBUF x
  tile shifted by k along time. TensorE does all MACs; ScalarE adds bias while
  evacuating PSUM; DMA overlaps via Tile double-buffering.
"""

import numpy as np

import concourse.bass as bass
import concourse.tile as tile
from concourse import mybir
from concourse.bass_utils import run_bass_kernel_spmd

B, T, C, K = 4, 4096, 2048, 7
NCORES = 8
CPC = C // NCORES  # 256 channels per core
NCB = CPC // 128   # 2 channel blocks of 128
TF = 512           # psum tile free size
NTT = T // TF      # 8 time tiles per (batch, channel-block)

F32 = mybir.dt.float32
# Matmul operand dtype tag. float32r is full-rate (1 cycle/row at N>=256)
# vs plain float32 at 1/4 rate; same 4-byte storage.
MM_DT = mybir.dt.float32r


def _split_multi_waits(nc, max_waits=1):
    """This container's walrus accepts at most ONE sync wait per instruction,
    but Tile emits several. Hoist extras onto single-wait NoOps inserted right
    before the instruction on the same engine (identical wait set at the same
    program point)."""
    uid = 0
    for f in nc.m.functions:
        for bb in f.blocks:
            new_insts = []
            for inst in bb.instructions:
                si = inst.sync_info
                if si is not None and si.on_wait and len(si.on_wait) > max_waits:
                    extra = si.on_wait[:-max_waits]
                    si.on_wait = si.on_wait[-max_waits:]
                    for w in extra:
                        uid += 1
                        nop = mybir.InstNoOp(name=f"{inst.name}_sw{uid}", ins=[], outs=[])
                        nop.engine = inst.engine
                        nop.sync_info = mybir.SyncInfo(on_wait=[w], on_update=[])
                        new_insts.append(nop)
                new_insts.append(inst)
            bb.instructions[:] = new_insts


def _build(repeat=1, variant="full", xbufs=3, xsplit=1):
    nc = bass.Bass("TRN2", target_bir_lowering=False)
    x = nc.dram_tensor("x", [B, CPC, T], MM_DT, kind="ExternalInput")
    wd = nc.dram_tensor("wd", [NCB, 128, K, 128], MM_DT, kind="ExternalInput")
    bias = nc.dram_tensor("bias", [NCB, 128, 1], F32, kind="ExternalInput")
    out = nc.dram_tensor("out", [B, CPC, T], F32, kind="ExternalOutput")

    with tile.TileContext(nc) as tc:
        with tc.tile_pool(name="singles", bufs=1) as singles, \
             tc.tile_pool(name="xp", bufs=xbufs) as xpool, \
             tc.tile_pool(name="op", bufs=2) as opool, \
             tc.tile_pool(name="ps", bufs=8, space="PSUM") as pspool:
            wd_sb = []
            bias_sb = []
            for cb in range(NCB):
                wt = singles.tile([128, K, 128], MM_DT, tag=f"wd{cb}")
                nc.sync.dma_start(out=wt, in_=wd[cb])
                bt = singles.tile([128, 1], F32, tag=f"b{cb}")
                nc.sync.dma_start(out=bt, in_=bias[cb])
                wd_sb.append(wt)
                bias_sb.append(bt)

            for _rep in range(repeat):
              for cb in range(NCB):
                for bi in range(B):
                    xt = xpool.tile([128, T + 6], MM_DT, tag="x")
                    nc.vector.memset(xt.bitcast(F32)[:, 0:6], 0.0)
                    tchunk = T // xsplit
                    for s in range(xsplit):
                        nc.sync.dma_start(
                            out=xt[:, 6 + s * tchunk: 6 + (s + 1) * tchunk],
                            in_=x[bi, cb * 128:(cb + 1) * 128,
                                  s * tchunk:(s + 1) * tchunk],
                        )
                    xr = xt
                    wr = wd_sb[cb]
                    if variant == "full":
                        ot = opool.tile([128, T], F32, tag="o")
                        for j in range(NTT):
                            ps = pspool.tile([128, TF], F32, tag="ps")
                            for k in range(K):
                                nc.tensor.matmul(
                                    ps,
                                    lhsT=wr[:, k, :],
                                    rhs=xr[:, j * TF + k: j * TF + k + TF],
                                    start=(k == 0),
                                    stop=(k == K - 1),
                                )
                            ocol = ot[:, j * TF:(j + 1) * TF]
                            if j % 2 == 0:
                                nc.scalar.add(out=ocol, in_=ps, add=bias_sb[cb])
                            else:
                                nc.vector.tensor_scalar_add(
                                    out=ocol, in0=ps, scalar1=bias_sb[cb]
                                )
                        nc.sync.dma_start(
                            out=out[bi, cb * 128:(cb + 1) * 128, :], in_=ot
                        )
                    elif variant == "v1":
                        for j in range(NTT):
                            ps = pspool.tile([128, TF], F32, tag="ps")
                            for k in range(K):
                                nc.tensor.matmul(
                                    ps,
                                    lhsT=wr[:, k, :],
                                    rhs=xr[:, j * TF + k: j * TF + k + TF],
                                    start=(k == 0),
                                    stop=(k == K - 1),
                                )
                            ot = opool.tile([128, TF], F32, tag="o1")
                            nc.scalar.add(out=ot, in_=ps, add=bias_sb[cb])
                            nc.sync.dma_start(
                                out=out[bi, cb * 128:(cb + 1) * 128, j * TF:(j + 1) * TF],
                                in_=ot,
                            )
                    elif variant == "pe_only":
                        ps = pspool.tile([128, TF], F32, tag="ps1")
                        for j in range(NTT):
                            for k in range(K):
                                nc.tensor.matmul(
                                    ps,
                                    lhsT=wr[:, k, :],
                                    rhs=xr[:, j * TF + k: j * TF + k + TF],
                                    start=(k == 0),
                                    stop=(k == K - 1),
                                )
                        last_ps = ps
            if variant == "pe_only":
                ot = opool.tile([128, TF], F32, tag="olast")
                nc.scalar.add(out=ot, in_=last_ps, add=bias_sb[0])
                nc.sync.dma_start(out=out[0, 0:128, 0:TF], in_=ot)

    _split_multi_waits(nc)
    return nc


_CACHE = {}


def _get_nc():
    if "nc" not in _CACHE:
        _CACHE["nc"] = _build()
    return _CACHE["nc"]


def build_in_maps(x, w, b):
    """Shard the full inputs into per-core input maps."""
    xT = np.ascontiguousarray(x.transpose(0, 2, 1)).astype(np.float32, copy=False)
    w2 = w.reshape(C, K).astype(np.float32, copy=False)
    b1 = b.astype(np.float32, copy=False)

    eye = np.eye(128, dtype=np.float32)
    in_maps = []
    for i in range(NCORES):
        c0 = i * CPC
        wcore = w2[c0:c0 + CPC]  # [CPC, K]
        # wd[cb, p, k, m] = (p == m) * w[c0 + cb*128 + p, k]
        wdm = eye[None, :, None, :] * wcore.reshape(NCB, 128, K)[:, :, :, None]
        in_maps.append({
            "x": np.ascontiguousarray(xT[:, c0:c0 + CPC, :]),
            "wd": np.ascontiguousarray(wdm.astype(np.float32)),
            "bias": np.ascontiguousarray(b1[c0:c0 + CPC].reshape(NCB, 128, 1)),
        })
    return in_maps


def kernel(x, w, b, **run_kwargs):
    """x: [B, T, C] f32; w: [C, 1, K] f32; b: [C] f32 -> [B, T, C] f32."""
    nc = _get_nc()
    in_maps = build_in_maps(x, w, b)
    res = run_bass_kernel_spmd(nc, in_maps, core_ids=list(range(NCORES)), **run_kwargs)

    outT = np.empty((B, C, T), dtype=np.float32)
    for i in range(NCORES):
        outT[:, i * CPC:(i + 1) * CPC, :] = res.results[i]["out"]
    out = np.ascontiguousarray(outT.transpose(0, 2, 1))
    if run_kwargs:
        return out, res
    return out
